# revision 46
# baseline (speedup 1.0000x reference)
"""Trainium2 Bass kernel for nn_LiquidS4Layer (S4 DPLR forward).

y = causal_conv(u, K) + D*u, with K the length-L SSM kernel computed from
small DPLR params (Lambda, P, B, C, step).

Algorithm (all on device, per core over 512 of the 4096 batch rows):
  1. Discretize via bilinear transform using the Woodbury identity
     (A = Lambda - P P^H is diagonal + rank-1, so (I - h A)^-1 is closed
     form): build block-real forms of Abar, Bbar.
  2. Alias-correct: the reference kernel is the *periodized* kernel
     K[l] = sum_m K_inf[l + m L]; equivalently apply (I - Abar^L)^-1
     (truncated Neumann series, Abar^L from repeated squaring).  The
     correction is applied on the C side (Wout'' = (I + X^T + X^2T) Wout,
     X = Abar^L, commutes with powers of Abar) so the B-side chains
     (V doubling -> Min/E) depend only on early squarings and overlap
     the squaring chain.
  3. Chunked convolution (chunk Q=128): per chunk, intra-chunk causal
     Toeplitz matmul with K[0:Q] (+ D on the diagonal), plus a rank-2N
     state passing: states h_i = Abar^Q h_{i-1} + Proj(u chunk i-1),
     far-field y += Re(Wout h_i).

Main loop is weight-stationary: the small [128,128] operator matrices
(T0 Toeplitz, Wout/W1 far-field maps, G0 direct map, Dq2T/E/Min state
maps) are the PE's stationary lhsT; u chunks and h stream through as
512-wide moving operands.  The h-update recurrence (trios) runs pairs
ahead of the near/far consumers.  The late squarings (k>=7) run in bf16
(their products only feed bf16 main-loop operators or the <=15% alias
correction).  Output is produced transposed ([t, batch]) and
un-transposed on the host.  u arrives pre-transposed/bf16 from the
host; y leaves as bf16.  Params arrive packed in one fp32 row.

Sharding: u/(y) row-sharded over 8 cores (batch*channel parallel); the
small params are replicated; no collectives.
"""
import os
import numpy as np
from contextlib import ExitStack

import ml_dtypes

import concourse.bass as bass
import concourse.tile as tile
from concourse import mybir
from concourse.bass_utils import run_bass_kernel_spmd

F32 = mybir.dt.float32
BF16 = mybir.dt.bfloat16

NCORES = 8
BH, L = 4096, 4096
BC = BH // NCORES       # 512 rows per core
N = 64                  # SSM state size
N2 = 2 * N              # real block state size = 128
Q = 128                 # chunk length
NCH = L // Q            # 32 chunks
NPAIR = NCH // 2

BF16_CHAIN = True       # late squarings (k>=7) in bf16

# packed param row offsets (par tensor, [1, PARW] fp32)
O_LRE, O_LIM, O_PRE, O_PIM = 0, 64, 128, 192
O_BRE, O_BIM, O_CRE, O_CIM = 256, 320, 384, 448
O_D, O_LSTEP, O_ONE = 512, 513, 514
PARW = 640

LAST_EXEC_NS = None
LAST_RESULTS = None


def _consts():
    ident = np.eye(128, dtype=np.float32)
    rev = ident[::-1].copy()                      # antidiagonal reversal
    ilmu = np.zeros((128, 128), dtype=np.float32)  # IL - IU blocks
    for p in range(64):
        ilmu[p, p + 64] = -1.0                    # -IU (top-right)
        ilmu[p + 64, p] = 1.0                     # +IL (bottom-left)
    cm = np.concatenate([ident, rev, ilmu], axis=1)     # [128, 384] fp32
    cmb = np.concatenate([ident, rev], axis=1).astype(ml_dtypes.bfloat16)
    return cm, cmb                                       # + [128, 256] bf16


def build_program():
    nc = bass.Bass()
    dp = nc.declare_dram_parameter
    ut = dp("ut", [128, NCH * BC], BF16, isOutput=False)   # [q, (i, j, b')]
    yt = dp("yt", [NCH, 128, BC], BF16, isOutput=True)     # [i, t, (j, b')]
    par = dp("par", [1, PARW], F32, isOutput=False)
    cmat = dp("cmat", [128, 384], F32, isOutput=False)
    cmatb = dp("cmatb", [128, 256], BF16, isOutput=False)

    with TileKernel(nc) as tk:
        tk.build(ut, yt, par, cmat, cmatb)
    _split_multi_waits(nc)
    return nc


def _split_multi_waits(nc):
    """This toolchain's walrus encodes at most one sync wait per (non-Drain)
    instruction.  Tile can emit several; hoist the extras onto standalone
    EventSemaphore wait instructions inserted just before, on the same
    engine (engines execute their stream in order, so this is equivalent)."""
    ctr = 0
    for f in nc.m.functions:
        for blk in f.blocks:
            out = []
            changed = False
            for inst in blk.instructions:
                si = inst.sync_info
                if si is None:
                    out.append(inst)
                    continue
                waits = list(si.on_wait)
                if len(waits) > 1:
                    # pick a non-DMA sem for the no-op update (the sim
                    # forbids foreign updates of in-flight DMA sems)
                    cands = [u for u in si.on_update] + [
                        w for w in waits if "DMA" not in w.ant_name]
                    for w in waits[:-1]:
                        ev = mybir.InstEventSemaphore(
                            name=f"I-wsplit-{ctr}", ins=[], outs=[])
                        ctr += 1
                        ev.engine = inst.engine
                        # zero-increment update: the sim requires >=1 update
                        # per instruction; +0 changes no semaphore value.
                        c = cands[0] if cands else w
                        up = mybir.SyncUpdate(
                            sync_type="semaphore", id=c.id, ant_name=c.ant_name,
                            update_mode="sem-add-imm", update_value=0,
                            update_reg=None)
                        ev.sync_info = mybir.SyncInfo(on_wait=[w], on_update=[up])
                        out.append(ev)
                    inst.sync_info = mybir.SyncInfo(
                        on_wait=[waits[-1]], on_update=list(si.on_update))
                    changed = True
                out.append(inst)
            if changed:
                blk.instructions = out


class TileKernel:
    def __init__(self, nc):
        self.nc = nc
        self.ctx = ExitStack()
        self.tc = tile.TileContext(nc)

    def __enter__(self):
        self.ctx.__enter__()
        self.tc.__enter__()
        return self

    def __exit__(self, *a):
        self.ctx.__exit__(*a)   # release pools before the scheduler runs
        return self.tc.__exit__(*a)

    def pool(self, name, bufs=1, space="SBUF"):
        return self.ctx.enter_context(
            self.tc.tile_pool(name=name, bufs=bufs, space=space))

    def build(self, ut_d, yt_d, par_d, cmat_d, cmatb_d):
        nc, tc = self.nc, self.tc
        con = self.pool("const", 1)
        pp = self.pool("pp", 1)          # param pipeline tiles (unique tags)
        pps = self.pool("pps", 2, "PSUM")
        dram = self.pool("dram", 1, "DRAM")

        def T(shape, dt=F32, p=pp, tag=None):
            return p.tile(shape, dt, tag=tag, name=tag)

        v = nc.vector
        s = nc.scalar
        g = nc.gpsimd

        # ---- act-table preload: dummy Exp on a zeroed tile (no deps) --
        zz = T([1, 1], tag="zz")
        v.memset(zz[:], 0.0)
        zze = T([1, 1], tag="zze")
        s.activation(zze[:], zz[:], mybir.ActivationFunctionType.Exp)

        # ---- load packed params & constants (2 DMAs) and u (4 DMAs) ---
        par = con.tile([1, PARW], F32, tag="par", name="par")
        nc.sync.dma_start(out=par[:], in_=par_d[:])
        cmt = con.tile([128, 384], F32, tag="cmt", name="cmt")
        nc.scalar.dma_start(out=cmt[:], in_=cmat_d[:])
        cmtb = con.tile([128, 256], BF16, tag="cmtb", name="cmtb")
        nc.scalar.dma_start(out=cmtb[:], in_=cmatb_d[:])

        ut_sb = con.tile([128, NCH * BC], BF16, tag="ut", name="ut")
        NPC = 4  # u load pieces
        W_PC = NCH * BC // NPC
        for pc in range(NPC):
            nc.gpsimd.dma_start(out=ut_sb[:, pc * W_PC:(pc + 1) * W_PC],
                                in_=ut_d[:, pc * W_PC:(pc + 1) * W_PC])

        def u_of(i):
            return ut_sb[:, i * BC:(i + 1) * BC]

        # zero prefix of the Toeplitz scratch (independent -> issue early)
        zs = dram.tile([256], BF16, tag="zscratch", name="zscratch")
        zrow = T([1, 128], dt=BF16, tag="zrow")
        v.memset(zrow[:], 0.0)
        nc.gpsimd.dma_start(out=zs[0:128], in_=zrow[:])

        lre = par[0:1, O_LRE:O_LRE + N]
        lim = par[0:1, O_LIM:O_LIM + N]
        pre = par[0:1, O_PRE:O_PRE + N]
        pim = par[0:1, O_PIM:O_PIM + N]
        bre = par[0:1, O_BRE:O_BRE + N]
        bim = par[0:1, O_BIM:O_BIM + N]
        cre = par[0:1, O_CRE:O_CRE + N]
        cimr = par[0:1, O_CIM:O_CIM + N]
        dval = par[0:1, O_D:O_D + 1]
        lstep = par[0:1, O_LSTEP:O_LSTEP + 1]
        one11 = par[0:1, O_ONE:O_ONE + 1]
        ident = cmt[:, 0:128]
        revm = cmt[:, 128:256]
        ilmu = cmt[:, 256:384]
        ident_b = cmtb[:, 0:128]
        revm_b = cmtb[:, 128:256]

        # ---- tiny param pipeline: spine on vector, side work on gpsimd
        # (gpsimd is SBUF-only, which all of these are)
        delta = T([1, 1], tag="delta")
        s.activation(delta[:], lstep, mybir.ActivationFunctionType.Exp)
        hh = T([1, 1], tag="hh")
        v.tensor_scalar_mul(hh[:], delta[:], 0.5)

        # independent-of-delta side work (gpsimd, starts as soon as par lands)
        p2 = T([1, N], tag="p2"); g1 = T([1, N], tag="g1"); g2 = T([1, N], tag="g2")
        g.tensor_mul(g1[:], pre, pre); g.tensor_mul(g2[:], pim, pim)
        g.tensor_add(p2[:], g1[:], g2[:])
        npim = T([1, N], tag="npim"); v.tensor_scalar_mul(npim[:], pim, -1.0)
        ncim = T([1, N], tag="ncim"); v.tensor_scalar_mul(ncim[:], cimr, -1.0)
        brow = T([1, 128], tag="brow")
        v.tensor_copy(brow[0:1, 0:N], bre)
        v.tensor_copy(brow[0:1, N:N2], bim)

        hlre = T([1, N], tag="hlre"); v.tensor_scalar_mul(hlre[:], lre, hh[:])
        hlim = T([1, N], tag="hlim"); v.tensor_scalar_mul(hlim[:], lim, hh[:])
        den_re = T([1, N], tag="den_re")
        v.tensor_scalar(den_re[:], hlre[:], -1.0, 1.0,
                        op0=mybir.AluOpType.mult, op1=mybir.AluOpType.add)
        den_im = T([1, N], tag="den_im")
        v.tensor_scalar_mul(den_im[:], hlim[:], -1.0)
        brow_s = T([1, 128], tag="brow_s")
        v.tensor_scalar_mul(brow_s[:], brow[:], delta[:])

        t1 = T([1, N], tag="t1"); t2 = T([1, N], tag="t2")
        r2 = T([1, N], tag="r2")
        v.tensor_mul(t1[:], den_re[:], den_re[:])
        v.tensor_mul(t2[:], den_im[:], den_im[:])
        v.tensor_add(r2[:], t1[:], t2[:])
        rinv = T([1, N], tag="rinv"); v.reciprocal(rinv[:], r2[:])
        d0re = T([1, N], tag="d0re"); v.tensor_mul(d0re[:], den_re[:], rinv[:])
        nden_im = T([1, N], tag="nden_im")
        v.tensor_scalar_mul(nden_im[:], den_im[:], -1.0)
        d0im = T([1, N], tag="d0im"); v.tensor_mul(d0im[:], nden_im[:], rinv[:])

        # s = 1 + h * sum(|P|^2 d0)  (complex)
        sr = T([1, 1], tag="sr"); si = T([1, 1], tag="si")
        v.tensor_mul(t1[:], p2[:], d0re[:])
        v.reduce_sum(sr[:], t1[:], axis=mybir.AxisListType.X)
        v.tensor_mul(t2[:], p2[:], d0im[:])
        v.reduce_sum(si[:], t2[:], axis=mybir.AxisListType.X)
        s_re = T([1, 1], tag="s_re")
        v.tensor_scalar(s_re[:], sr[:], hh[:], 1.0,
                        op0=mybir.AluOpType.mult, op1=mybir.AluOpType.add)
        s_im = T([1, 1], tag="s_im"); v.tensor_mul(s_im[:], si[:], hh[:])
        # hs = h / s  (complex)
        s2 = T([1, 1], tag="s2"); sa = T([1, 1], tag="sa"); sb = T([1, 1], tag="sb")
        v.tensor_mul(sa[:], s_re[:], s_re[:]); v.tensor_mul(sb[:], s_im[:], s_im[:])
        v.tensor_add(s2[:], sa[:], sb[:])
        s2i = T([1, 1], tag="s2i"); v.reciprocal(s2i[:], s2[:])
        hs_re = T([1, 1], tag="hs_re"); hs_im = T([1, 1], tag="hs_im")
        v.tensor_mul(sa[:], s_re[:], s2i[:]); v.tensor_mul(hs_re[:], sa[:], hh[:])
        v.tensor_mul(sb[:], s_im[:], s2i[:]); v.tensor_mul(sa[:], sb[:], hh[:])
        v.tensor_scalar_mul(hs_im[:], sa[:], -1.0)

        # t = d0*P ; w = hs * t ; vv = conj(P)*d0
        tre = T([1, N], tag="tre"); tim = T([1, N], tag="tim")
        v.tensor_mul(t1[:], d0re[:], pre); v.tensor_mul(t2[:], d0im[:], pim)
        v.tensor_sub(tre[:], t1[:], t2[:])
        v.tensor_mul(t1[:], d0re[:], pim); v.tensor_mul(t2[:], d0im[:], pre)
        v.tensor_add(tim[:], t1[:], t2[:])
        vre = T([1, N], tag="vre"); vim = T([1, N], tag="vim")
        v.tensor_mul(t1[:], pre, d0re[:]); v.tensor_mul(t2[:], pim, d0im[:])
        v.tensor_add(vre[:], t1[:], t2[:])
        v.tensor_mul(t1[:], pre, d0im[:]); v.tensor_mul(t2[:], pim, d0re[:])
        v.tensor_sub(vim[:], t1[:], t2[:])
        wre = T([1, N], tag="wre"); wim = T([1, N], tag="wim")
        v.tensor_scalar_mul(t1[:], tre[:], hs_re[:])
        v.tensor_scalar_mul(t2[:], tim[:], hs_im[:])
        v.tensor_sub(wre[:], t1[:], t2[:])
        v.tensor_scalar_mul(t1[:], tim[:], hs_re[:])
        v.tensor_scalar_mul(t2[:], tre[:], hs_im[:])
        v.tensor_add(wim[:], t1[:], t2[:])

        # ---- block matrices blkA1 = block(A1), blkA0H = block(A0inv^H)
        def rowcat(tag, left, right, eng=None):
            rt = T([1, 128], tag=tag)
            (eng or v).tensor_copy(rt[0:1, 0:N], left)
            (eng or v).tensor_copy(rt[0:1, N:N2], right)
            return rt

        def blk_from_parts(tag, gre, gim, a_rows, b_rows):
            # block(diag(g)) - block(outer(a,b)); a_rows/b_rows are two
            # [1,128] rows each: block(outer) = sum_r a_rows[r]^T b_rows[r]
            ps = pps.tile([128, 128], F32, tag="pp_ps", name="ps_blk")
            nc.tensor.matmul(ps[:], a_rows[0][:], b_rows[0][:],
                             start=True, stop=False)
            nc.tensor.matmul(ps[:], a_rows[1][:], b_rows[1][:],
                             start=False, stop=True)
            grow = rowcat(tag + "_grow", gre, gre)
            girow = rowcat(tag + "_girow", gim, gim)
            ps2 = pps.tile([128, 2], F32, tag="pp_ps", name="ps_g2")
            nc.tensor.matmul(ps2[:, 0:1], grow[:], one11, start=True, stop=True)
            nc.tensor.matmul(ps2[:, 1:2], girow[:], one11, start=True, stop=True)
            gcol = T([128, 1], tag=tag + "_gcol")
            gicol = T([128, 1], tag=tag + "_gicol")
            v.tensor_copy(gcol[:], ps2[:, 0:1])
            v.tensor_copy(gicol[:], ps2[:, 1:2])
            dg = T([128, 128], tag=tag + "_dg")
            v.tensor_scalar_mul(dg[:], ident, gcol[:])
            dgi = T([128, 128], tag=tag + "_dgi")
            v.tensor_scalar_mul(dgi[:], ilmu, gicol[:])
            out = T([128, 128], tag=tag)
            v.tensor_add(out[:], dg[:], dgi[:])
            v.tensor_sub(out[:], out[:], ps[:])
            return out

        g1re = T([1, N], tag="g1re")
        v.tensor_scalar_add(g1re[:], hlre[:], 1.0)
        a1re = T([1, N], tag="a1re"); v.tensor_scalar_mul(a1re[:], pre, hh[:])
        a1im = T([1, N], tag="a1im"); v.tensor_scalar_mul(a1im[:], pim, hh[:])
        na1im = T([1, N], tag="na1im"); v.tensor_scalar_mul(na1im[:], a1im[:], -1.0)
        a1_rows = [rowcat("a1r0", a1re[:], a1im[:]),
                   rowcat("a1r1", na1im[:], a1re[:])]
        b1_rows = [rowcat("b1r0", pre, pim), rowcat("b1r1", npim[:], pre)]
        blkA1 = blk_from_parts("blkA1", g1re[:], hlim[:], a1_rows, b1_rows)

        nd0im = T([1, N], tag="nd0im"); v.tensor_scalar_mul(nd0im[:], d0im[:], -1.0)
        nvim = T([1, N], tag="nvim"); v.tensor_scalar_mul(nvim[:], vim[:], -1.0)
        nwim = T([1, N], tag="nwim"); v.tensor_scalar_mul(nwim[:], wim[:], -1.0)
        a0_rows = [rowcat("a0r0", vre[:], nvim[:]), rowcat("a0r1", vim[:], vre[:])]
        b0_rows = [rowcat("b0r0", wre[:], wim[:]), rowcat("b0r1", nwim[:], wre[:])]
        blkA0H = blk_from_parts("blkA0H", d0re[:], nd0im[:], a0_rows, b0_rows)

        # ---- helpers -------------------------------------------------
        def ev_copy(eng, dst, src):
            if eng is s:
                s.copy(dst, src)
            else:
                eng.tensor_copy(dst, src)

        def mm_ev(lhsT, rhs, m, n_, tag, eng=None, dt=F32):
            ps = pps.tile([128, max(n_, 1)], F32, tag="pp_ps", name="ps_mm")
            nc.tensor.matmul(ps[0:m, 0:n_], lhsT[:], rhs[:], start=True, stop=True)
            t = T([m, n_], dt=dt, tag=tag)
            ev_copy(eng or v, t[:], ps[0:m, 0:n_])
            return t

        cast = con.tile

        def bf(name, srcf, eng=None):
            t = cast([128, 128], BF16, tag=name, name=name)
            ev_copy(eng or v, t[:], srcf[:])
            return t

        # ---- Bbar: b2 = block(A0inv) @ (delta*B)  (V chain needs it) -
        psb = pps.tile([128, 1], F32, tag="pp_ps", name="ps_bcol")
        nc.tensor.matmul(psb[:, 0:1], brow_s[:], one11, start=True, stop=True)
        bcol = T([128, 1], tag="bcol")
        v.tensor_copy(bcol[:], psb[:, 0:1])
        b2 = mm_ev(blkA0H, bcol, 128, 1, "b2")

        # ---- Abar pair + squaring chain (bf16 throughout) ------------
        NSQ = 12   # Abar^(2^12) = Abar^4096
        A2 = [None] * (NSQ + 1)
        A2T = [None] * (NSQ + 1)
        A2[0] = mm_ev(blkA0H, blkA1, 128, 128, "A2_0")
        A2T[0] = mm_ev(blkA1, blkA0H, 128, 128, "A2T_0", eng=s)

        def sq_pair(k, need_at=True, bf16=False):
            dt_out = BF16 if bf16 else F32
            ps = pps.tile([128, 256], F32, tag="pp_ps", name="sqpair")
            nc.tensor.matmul(ps[:, 0:128], A2T[k][:], A2[k][:],
                             start=True, stop=(not need_at))
            if need_at:
                nc.tensor.matmul(ps[:, 128:256], A2[k][:], A2T[k][:],
                                 start=False, stop=True)
            a = T([128, 128], dt=dt_out, tag=f"A2_{k+1}")
            v.tensor_copy(a[:], ps[:, 0:128])
            at = None
            if need_at:
                at = T([128, 128], dt=dt_out, tag=f"A2T_{k+1}")
                s.copy(at[:], ps[:, 128:256])
            return a, at

        # Ccol chain and V chain interleave with the squaring chain; the
        # dataflow scheduler runs them in the chain's gaps.
        c0row = rowcat("c0row", cre, cimr)
        c1row = rowcat("c1row", ncim[:], cre)
        Ccol = T([128, 128], tag="Ccol")
        psc = pps.tile([128, 128], F32, tag="pp_ps", name="psc")
        nc.tensor.matmul(psc[:, 0:1], c0row[:], one11, start=True, stop=False)
        nc.tensor.matmul(psc[:, 1:2], c1row[:], one11, start=False, stop=True)
        v.tensor_copy(Ccol[:, 0:2], psc[:, 0:2])

        V = T([128, 128], tag="Vd")
        psv0 = pps.tile([128, 128], F32, tag="pp_ps", name="psv0")
        nc.tensor.matmul(psv0[:, 0:1], A2T[0][:], b2[:], start=True, stop=True)
        s.copy(V[:, 0:1], psv0[:, 0:1])

        def ccol_step(k):
            nr = 2 << k
            psr = pps.tile([128, 128], F32, tag="pp_ps", name="psr")
            nc.tensor.matmul(psr[:, 0:nr], A2[k][:], Ccol[:, 0:nr],
                             start=True, stop=True)
            v.tensor_copy(Ccol[:, nr:2 * nr], psr[:, 0:nr])

        def v_step(k):
            wd = 1 << k
            psv = pps.tile([128, 128], F32, tag="pp_ps", name="psv")
            nc.tensor.matmul(psv[:, 0:wd], A2T[k][:], V[:, 0:wd],
                             start=True, stop=True)
            s.copy(V[:, wd:2 * wd], psv[:, 0:wd])

        for k in range(7):
            A2[k + 1], A2T[k + 1] = sq_pair(k)
            if k > 0:
                v_step(k - 1)
                ccol_step(k - 1)

        v_step(5); v_step(6)
        Ccol_hi = mm_ev(A2[6], Ccol, 128, 128, "Ccol_hi")
        Wout_b = T([128, 128], tag="Wout_b")
        v.tensor_copy(Wout_b[:, 0:64], Ccol[:, 0:128:2])
        v.tensor_copy(Wout_b[:, 64:128], Ccol_hi[:, 0:128:2])

        # W = V^T via PE (fp32 transpose of the shadow); Min = rev @ W ;
        # MinT = V @ rev -- evicted straight to bf16
        psb_p = self.pool("psb", 1, "PSUM")
        psw = psb_p.tile([128, 128], F32, tag="psw", name="psw")
        nc.tensor.transpose(psw[:], V[:], ident)
        W = T([128, 128], tag="Wd")
        v.tensor_copy(W[:], psw[:])
        Min_bf = mm_ev(revm, W, 128, 128, "Min_bf", eng=s, dt=BF16)
        MinT_b = mm_ev(W, revm, 128, 128, "MinT_b")
        E_bf = mm_ev(MinT_b, A2T[7], 128, 128, "E_bf", eng=s, dt=BF16)

        A2_7f, A2T_7f = A2[7], A2T[7]
        A2[7] = bf("A2b7", A2[7])
        A2T[7] = bf("A2Tb7", A2T[7], eng=s)
        for k in range(7, NSQ):
            A2[k + 1], A2T[k + 1] = sq_pair(k, need_at=(k < NSQ - 1), bf16=True)
        Dq2T_bf = A2T[8]

        # ================= main loop setup ============================
        hp = self.pool("h", 3)
        yp = self.pool("yt", 4)
        ph_p = self.pool("ph", 2, "PSUM")
        py_p = self.pool("py", 3, "PSUM")
        yt_r = yt_d.rearrange("i t b -> t i b")

        h_tiles = [None] * NPAIR

        def trio(k):
            # h_k = Dq2 h_{k-1} + E u_{2k-2} + Min u_{2k-1}
            ph = ph_p.tile([128, BC], F32, tag="ph", name="ph")
            first = True
            if h_tiles[k - 1] is not None:
                nc.tensor.matmul(ph[:], Dq2T_bf[:], h_tiles[k - 1][:],
                                 start=True, stop=False)
                first = False
            nc.tensor.matmul(ph[:], E_bf[:], u_of(2 * k - 2),
                             start=first, stop=False)
            nc.tensor.matmul(ph[:], Min_bf[:], u_of(2 * k - 1),
                             start=False, stop=True)
            h_cur = hp.tile([128, BC], BF16, tag="h", name="h")
            ev_copy(v if k % 2 else s, h_cur[:], ph[:])
            h_tiles[k] = h_cur

        def consumer(j):
            # far-field first (T0 last) so the T0 tail doesn't stall PE
            h_j = h_tiles[j]
            py_e = py_p.tile([128, BC], F32, tag="py", name="py_e")
            if h_j is not None:
                nc.tensor.matmul(py_e[:], Wout_bf[:], h_j[:],
                                 start=True, stop=False)
                nc.tensor.matmul(py_e[:], T0_bf[:], u_of(2 * j),
                                 start=False, stop=True)
            else:
                nc.tensor.matmul(py_e[:], T0_bf[:], u_of(2 * j),
                                 start=True, stop=True)
            yt_e = yp.tile([128, BC], BF16, tag="ytt", name="yt_e")
            ev_copy(v, yt_e[:], py_e[:])
            nc.sync.dma_start(out=yt_r[:, 2 * j, :], in_=yt_e[:])
            py_o = py_p.tile([128, BC], F32, tag="py", name="py_o")
            nc.tensor.matmul(py_o[:], G0_bf[:], u_of(2 * j),
                             start=True, stop=False)
            if h_j is not None:
                nc.tensor.matmul(py_o[:], W1_bf[:], h_j[:],
                                 start=False, stop=False)
            nc.tensor.matmul(py_o[:], T0_bf[:], u_of(2 * j + 1),
                             start=False, stop=True)
            yt_o = yp.tile([128, BC], BF16, tag="ytt", name="yt_o")
            ev_copy(s, yt_o[:], py_o[:])
            nc.gpsimd.dma_start(out=yt_r[:, 2 * j + 1, :], in_=yt_o[:])

        # trios can pre-run as soon as Min/E/Dq2 are ready
        trio(1); trio(2)

        # ---- alias correction on the C side: Wout'' = (I+X^T+X^2T)Wout
        Wout_pre_bf = bf("Wout_pre_bf", Wout_b)
        wc1 = mm_ev(A2[NSQ], Wout_pre_bf, 128, 128, "wc1")
        trio(3)
        wc1b = bf("wc1b", wc1)
        wc2 = mm_ev(A2[NSQ], wc1b, 128, 128, "wc2", eng=s)
        v.tensor_add(Wout_b[:], Wout_b[:], wc1[:])
        v.tensor_add(Wout_b[:], Wout_b[:], wc2[:])
        Wout_bf = bf("Wout_bf", Wout_b)
        trio(4)

        # ---- K taps row -> Toeplitz T0 via DRAM shift trick ----------
        psk = pps.tile([128, 128], F32, tag="pp_ps", name="psk")
        nc.tensor.matmul(psk[0:1, 0:128], b2[:], Wout_b[:], start=True, stop=True)
        zK = T([1, 128], tag="zK")
        v.tensor_copy(zK[:], psk[0:1, 0:128])
        v.tensor_add(zK[0:1, 0:1], zK[0:1, 0:1], dval)   # += D at lag 0
        zKb = T([1, 128], dt=BF16, tag="zKb")
        v.tensor_copy(zKb[:], zK[:])
        nc.sync.dma_start(out=zs[128:256], in_=zKb[:])
        trio(5)
        # T0R[p, t] = zs[1 + p + t] = T0[127-p, t]; un-reverse via rev@T0R
        T0R = T([128, 128], dt=BF16, tag="T0R")
        zsap = zs[:]
        src = bass.AP(zsap.tensor, zsap.offset + 1, [[1, 128], [1, 128]])
        nc.sync.dma_start(out=T0R[:], in_=src)
        # remaining far maps while the DMAs fly:
        # W1 = block(Dq)^T @ Wout'' ; G0 = Min @ Wout''
        W1_bf = mm_ev(A2_7f, Wout_b, 128, 128, "W1_bf", dt=BF16)
        G0_bf = mm_ev(MinT_b, Wout_b, 128, 128, "G0_bf", eng=s, dt=BF16)
        trio(6)
        T0_bf = mm_ev(revm_b, T0R, 128, 128, "T0_bf", dt=BF16)
        trio(7)

        # ================= main loop ==================================
        nxt = 0
        for it in range(8, NPAIR):
            trio(it)
            consumer(nxt); nxt += 1
        while nxt < NPAIR:
            consumer(nxt); nxt += 1


def kernel(**inputs):
    global LAST_EXEC_NS, LAST_RESULTS
    nc = build_program()
    cmat, cmatb = _consts()

    u = np.asarray(inputs["u"], dtype=np.float32)
    # per-core pre-transpose to [q, i, (j, b')] and cast to bf16
    # row = c*512 + j*128 + b', col = i*128 + q
    ut = u.reshape(NCORES, 4, 128, NCH, 128).transpose(0, 4, 3, 1, 2)
    ut = np.ascontiguousarray(ut).reshape(NCORES, 128, NCH * BC)
    ut = ut.astype(ml_dtypes.bfloat16)

    par = np.zeros((1, PARW), dtype=np.float32)
    par[0, O_LRE:O_LRE + N] = np.asarray(inputs["Lambda_re"], np.float32)
    par[0, O_LIM:O_LIM + N] = np.asarray(inputs["Lambda_im"], np.float32)
    par[0, O_PRE:O_PRE + N] = np.asarray(inputs["P_re"], np.float32)
    par[0, O_PIM:O_PIM + N] = np.asarray(inputs["P_im"], np.float32)
    par[0, O_BRE:O_BRE + N] = np.asarray(inputs["B_re"], np.float32)
    par[0, O_BIM:O_BIM + N] = np.asarray(inputs["B_im"], np.float32)
    par[0, O_CRE:O_CRE + N] = np.asarray(inputs["C_ri"], np.float32)[:, 0]
    par[0, O_CIM:O_CIM + N] = np.asarray(inputs["C_ri"], np.float32)[:, 1]
    par[0, O_D] = np.asarray(inputs["D"], np.float32).reshape(-1)[0]
    par[0, O_LSTEP] = np.asarray(inputs["log_step"], np.float32).reshape(-1)[0]
    par[0, O_ONE] = 1.0

    in_maps = []
    for c in range(NCORES):
        in_maps.append({"ut": ut[c], "par": par, "cmat": cmat, "cmatb": cmatb})

    trace = bool(int(os.environ.get("KERNEL_TRACE", "0")))
    kw = {}
    if trace:
        kw["trace"] = True
        kw["trace_cores"] = list(range(NCORES))
    res = run_bass_kernel_spmd(nc, in_maps, list(range(NCORES)), **kw)
    LAST_EXEC_NS = res.exec_time_ns
    LAST_RESULTS = res

    y = np.empty((BH, L), dtype=np.float32)
    for c in range(NCORES):
        ytc = np.asarray(res.results[c]["yt"]).astype(np.float32)  # [i, t, jb]
        y[c * BC:(c + 1) * BC] = (
            ytc.reshape(NCH, 128, 4, 128).transpose(2, 3, 0, 1).reshape(BC, L))
    return y


# revision 47
# speedup vs baseline: 1.0375x; 1.0375x over previous
"""Trainium2 Bass kernel for nn_LiquidS4Layer (S4 DPLR forward).

y = causal_conv(u, K) + D*u, with K the length-L SSM kernel computed from
small DPLR params (Lambda, P, B, C, step).

Algorithm (all on device, per core over 512 of the 4096 batch rows):
  1. Discretize via bilinear transform using the Woodbury identity
     (A = Lambda - P P^H is diagonal + rank-1, so (I - h A)^-1 is closed
     form): build block-real forms of Abar, Bbar.
  2. Alias-correct: the reference kernel is the *periodized* kernel
     K[l] = sum_m K_inf[l + m L]; equivalently apply (I - Abar^L)^-1
     (truncated Neumann series, Abar^L from repeated squaring).  The
     correction is applied on the C side (Wout'' = (I + X^T + X^2T) Wout,
     X = Abar^L, commutes with powers of Abar) so the B-side chains
     (V doubling -> Min/E) depend only on early squarings and overlap
     the squaring chain.
  3. Chunked convolution (chunk Q=128): per chunk, intra-chunk causal
     Toeplitz matmul with K[0:Q] (+ D on the diagonal), plus a rank-2N
     state passing: states h_i = Abar^Q h_{i-1} + Proj(u chunk i-1),
     far-field y += Re(Wout h_i).

Main loop is weight-stationary: the small [128,128] operator matrices
(T0 Toeplitz, Wout/W1 far-field maps, G0 direct map, Dq2T/E/Min state
maps) are the PE's stationary lhsT; u chunks and h stream through as
512-wide moving operands.  The h-update recurrence (trios) runs pairs
ahead of the near/far consumers.  The late squarings (k>=7) run in bf16
(their products only feed bf16 main-loop operators or the <=15% alias
correction).  Output is produced transposed ([t, batch]) and
un-transposed on the host.  u arrives pre-transposed/bf16 from the
host; y leaves as bf16.  Params arrive packed in one fp32 row.

Sharding: u/(y) row-sharded over 8 cores (batch*channel parallel); the
small params are replicated; no collectives.
"""
import os
import numpy as np
from contextlib import ExitStack

import ml_dtypes

import concourse.bass as bass
import concourse.tile as tile
from concourse import mybir
from concourse.bass_utils import run_bass_kernel_spmd

F32 = mybir.dt.float32
BF16 = mybir.dt.bfloat16

NCORES = 8
BH, L = 4096, 4096
BC = BH // NCORES       # 512 rows per core
N = 64                  # SSM state size
N2 = 2 * N              # real block state size = 128
Q = 128                 # chunk length
NCH = L // Q            # 32 chunks
NPAIR = NCH // 2

BF16_CHAIN = True       # late squarings (k>=7) in bf16

# packed param row offsets (par tensor, [1, PARW] fp32)
O_LRE, O_LIM, O_PRE, O_PIM = 0, 64, 128, 192
O_BRE, O_BIM, O_CRE, O_CIM = 256, 320, 384, 448
O_D, O_LSTEP, O_ONE = 512, 513, 514
PARW = 640

LAST_EXEC_NS = None
LAST_RESULTS = None


def _consts():
    ident = np.eye(128, dtype=np.float32)
    rev = ident[::-1].copy()                      # antidiagonal reversal
    ilmu = np.zeros((128, 128), dtype=np.float32)  # IL - IU blocks
    for p in range(64):
        ilmu[p, p + 64] = -1.0                    # -IU (top-right)
        ilmu[p + 64, p] = 1.0                     # +IL (bottom-left)
    return np.concatenate([ident, rev, ilmu], axis=1)  # [128, 384]


def build_program():
    nc = bass.Bass()
    dp = nc.declare_dram_parameter
    ut = dp("ut", [128, NCH * BC], BF16, isOutput=False)   # [q, (i, j, b')]
    yt = dp("yt", [NCH, 128, BC], BF16, isOutput=True)     # [i, t, (j, b')]
    par = dp("par", [1, PARW], F32, isOutput=False)
    cmat = dp("cmat", [128, 384], F32, isOutput=False)

    with TileKernel(nc) as tk:
        tk.build(ut, yt, par, cmat)
    _split_multi_waits(nc)
    return nc


def _split_multi_waits(nc):
    """This toolchain's walrus encodes at most one sync wait per (non-Drain)
    instruction.  Tile can emit several; hoist the extras onto standalone
    EventSemaphore wait instructions inserted just before, on the same
    engine (engines execute their stream in order, so this is equivalent)."""
    ctr = 0
    for f in nc.m.functions:
        for blk in f.blocks:
            out = []
            changed = False
            for inst in blk.instructions:
                si = inst.sync_info
                if si is None:
                    out.append(inst)
                    continue
                waits = list(si.on_wait)
                if len(waits) > 1:
                    # pick a non-DMA sem for the no-op update (the sim
                    # forbids foreign updates of in-flight DMA sems)
                    cands = [u for u in si.on_update] + [
                        w for w in waits if "DMA" not in w.ant_name]
                    for w in waits[:-1]:
                        ev = mybir.InstEventSemaphore(
                            name=f"I-wsplit-{ctr}", ins=[], outs=[])
                        ctr += 1
                        ev.engine = inst.engine
                        # zero-increment update: the sim requires >=1 update
                        # per instruction; +0 changes no semaphore value.
                        c = cands[0] if cands else w
                        up = mybir.SyncUpdate(
                            sync_type="semaphore", id=c.id, ant_name=c.ant_name,
                            update_mode="sem-add-imm", update_value=0,
                            update_reg=None)
                        ev.sync_info = mybir.SyncInfo(on_wait=[w], on_update=[up])
                        out.append(ev)
                    inst.sync_info = mybir.SyncInfo(
                        on_wait=[waits[-1]], on_update=list(si.on_update))
                    changed = True
                out.append(inst)
            if changed:
                blk.instructions = out


class TileKernel:
    def __init__(self, nc):
        self.nc = nc
        self.ctx = ExitStack()
        self.tc = tile.TileContext(nc)

    def __enter__(self):
        self.ctx.__enter__()
        self.tc.__enter__()
        return self

    def __exit__(self, *a):
        self.ctx.__exit__(*a)   # release pools before the scheduler runs
        return self.tc.__exit__(*a)

    def pool(self, name, bufs=1, space="SBUF"):
        return self.ctx.enter_context(
            self.tc.tile_pool(name=name, bufs=bufs, space=space))

    def build(self, ut_d, yt_d, par_d, cmat_d):
        nc, tc = self.nc, self.tc
        con = self.pool("const", 1)
        pp = self.pool("pp", 1)          # param pipeline tiles (unique tags)
        pps = self.pool("pps", 2, "PSUM")
        dram = self.pool("dram", 1, "DRAM")

        def T(shape, dt=F32, p=pp, tag=None):
            return p.tile(shape, dt, tag=tag, name=tag)

        v = nc.vector
        s = nc.scalar
        g = nc.gpsimd

        # ---- act-table preload: dummy Exp on a zeroed tile (no deps) --
        zz = T([1, 1], tag="zz")
        v.memset(zz[:], 0.0)
        zze = T([1, 1], tag="zze")
        s.activation(zze[:], zz[:], mybir.ActivationFunctionType.Exp)

        # ---- load packed params & constants (2 DMAs) and u (4 DMAs) ---
        par = con.tile([1, PARW], F32, tag="par", name="par")
        nc.sync.dma_start(out=par[:], in_=par_d[:])
        cmt = con.tile([128, 384], F32, tag="cmt", name="cmt")
        nc.scalar.dma_start(out=cmt[:], in_=cmat_d[:])

        ut_sb = con.tile([128, NCH * BC], BF16, tag="ut", name="ut")
        NPC = 4  # u load pieces
        W_PC = NCH * BC // NPC
        for pc in range(NPC):
            nc.gpsimd.dma_start(out=ut_sb[:, pc * W_PC:(pc + 1) * W_PC],
                                in_=ut_d[:, pc * W_PC:(pc + 1) * W_PC])

        def u_of(i):
            return ut_sb[:, i * BC:(i + 1) * BC]

        # zero prefix of the Toeplitz scratch (independent -> issue early)
        zs = dram.tile([256], F32, tag="zscratch", name="zscratch")
        zrow = T([1, 128], tag="zrow")
        v.memset(zrow[:], 0.0)
        nc.gpsimd.dma_start(out=zs[0:128], in_=zrow[:])

        lre = par[0:1, O_LRE:O_LRE + N]
        lim = par[0:1, O_LIM:O_LIM + N]
        pre = par[0:1, O_PRE:O_PRE + N]
        pim = par[0:1, O_PIM:O_PIM + N]
        bre = par[0:1, O_BRE:O_BRE + N]
        bim = par[0:1, O_BIM:O_BIM + N]
        cre = par[0:1, O_CRE:O_CRE + N]
        cimr = par[0:1, O_CIM:O_CIM + N]
        dval = par[0:1, O_D:O_D + 1]
        lstep = par[0:1, O_LSTEP:O_LSTEP + 1]
        one11 = par[0:1, O_ONE:O_ONE + 1]
        ident = cmt[:, 0:128]
        revm = cmt[:, 128:256]
        ilmu = cmt[:, 256:384]

        # ---- tiny param pipeline: spine on vector, side work on gpsimd
        # (gpsimd is SBUF-only, which all of these are)
        delta = T([1, 1], tag="delta")
        s.activation(delta[:], lstep, mybir.ActivationFunctionType.Exp)
        hh = T([1, 1], tag="hh")
        v.tensor_scalar_mul(hh[:], delta[:], 0.5)

        # independent-of-delta side work (gpsimd, starts as soon as par lands)
        p2 = T([1, N], tag="p2"); g1 = T([1, N], tag="g1"); g2 = T([1, N], tag="g2")
        g.tensor_mul(g1[:], pre, pre); g.tensor_mul(g2[:], pim, pim)
        g.tensor_add(p2[:], g1[:], g2[:])
        npim = T([1, N], tag="npim"); v.tensor_scalar_mul(npim[:], pim, -1.0)
        ncim = T([1, N], tag="ncim"); v.tensor_scalar_mul(ncim[:], cimr, -1.0)
        brow = T([1, 128], tag="brow")
        v.tensor_copy(brow[0:1, 0:N], bre)
        v.tensor_copy(brow[0:1, N:N2], bim)

        hlre = T([1, N], tag="hlre"); v.tensor_scalar_mul(hlre[:], lre, hh[:])
        hlim = T([1, N], tag="hlim"); v.tensor_scalar_mul(hlim[:], lim, hh[:])
        den_re = T([1, N], tag="den_re")
        v.tensor_scalar(den_re[:], hlre[:], -1.0, 1.0,
                        op0=mybir.AluOpType.mult, op1=mybir.AluOpType.add)
        den_im = T([1, N], tag="den_im")
        v.tensor_scalar_mul(den_im[:], hlim[:], -1.0)
        brow_s = T([1, 128], tag="brow_s")
        v.tensor_scalar_mul(brow_s[:], brow[:], delta[:])

        t1 = T([1, N], tag="t1"); t2 = T([1, N], tag="t2")
        r2 = T([1, N], tag="r2")
        v.tensor_mul(t1[:], den_re[:], den_re[:])
        v.tensor_mul(t2[:], den_im[:], den_im[:])
        v.tensor_add(r2[:], t1[:], t2[:])
        rinv = T([1, N], tag="rinv"); v.reciprocal(rinv[:], r2[:])
        d0re = T([1, N], tag="d0re"); v.tensor_mul(d0re[:], den_re[:], rinv[:])
        nden_im = T([1, N], tag="nden_im")
        v.tensor_scalar_mul(nden_im[:], den_im[:], -1.0)
        d0im = T([1, N], tag="d0im"); v.tensor_mul(d0im[:], nden_im[:], rinv[:])

        # s = 1 + h * sum(|P|^2 d0)  (complex)
        sr = T([1, 1], tag="sr"); si = T([1, 1], tag="si")
        v.tensor_mul(t1[:], p2[:], d0re[:])
        v.reduce_sum(sr[:], t1[:], axis=mybir.AxisListType.X)
        v.tensor_mul(t2[:], p2[:], d0im[:])
        v.reduce_sum(si[:], t2[:], axis=mybir.AxisListType.X)
        s_re = T([1, 1], tag="s_re")
        v.tensor_scalar(s_re[:], sr[:], hh[:], 1.0,
                        op0=mybir.AluOpType.mult, op1=mybir.AluOpType.add)
        s_im = T([1, 1], tag="s_im"); v.tensor_mul(s_im[:], si[:], hh[:])
        # hs = h / s  (complex)
        s2 = T([1, 1], tag="s2"); sa = T([1, 1], tag="sa"); sb = T([1, 1], tag="sb")
        v.tensor_mul(sa[:], s_re[:], s_re[:]); v.tensor_mul(sb[:], s_im[:], s_im[:])
        v.tensor_add(s2[:], sa[:], sb[:])
        s2i = T([1, 1], tag="s2i"); v.reciprocal(s2i[:], s2[:])
        hs_re = T([1, 1], tag="hs_re"); hs_im = T([1, 1], tag="hs_im")
        v.tensor_mul(sa[:], s_re[:], s2i[:]); v.tensor_mul(hs_re[:], sa[:], hh[:])
        v.tensor_mul(sb[:], s_im[:], s2i[:]); v.tensor_mul(sa[:], sb[:], hh[:])
        v.tensor_scalar_mul(hs_im[:], sa[:], -1.0)

        # t = d0*P ; w = hs * t ; vv = conj(P)*d0
        tre = T([1, N], tag="tre"); tim = T([1, N], tag="tim")
        v.tensor_mul(t1[:], d0re[:], pre); v.tensor_mul(t2[:], d0im[:], pim)
        v.tensor_sub(tre[:], t1[:], t2[:])
        v.tensor_mul(t1[:], d0re[:], pim); v.tensor_mul(t2[:], d0im[:], pre)
        v.tensor_add(tim[:], t1[:], t2[:])
        vre = T([1, N], tag="vre"); vim = T([1, N], tag="vim")
        v.tensor_mul(t1[:], pre, d0re[:]); v.tensor_mul(t2[:], pim, d0im[:])
        v.tensor_add(vre[:], t1[:], t2[:])
        v.tensor_mul(t1[:], pre, d0im[:]); v.tensor_mul(t2[:], pim, d0re[:])
        v.tensor_sub(vim[:], t1[:], t2[:])
        wre = T([1, N], tag="wre"); wim = T([1, N], tag="wim")
        v.tensor_scalar_mul(t1[:], tre[:], hs_re[:])
        v.tensor_scalar_mul(t2[:], tim[:], hs_im[:])
        v.tensor_sub(wre[:], t1[:], t2[:])
        v.tensor_scalar_mul(t1[:], tim[:], hs_re[:])
        v.tensor_scalar_mul(t2[:], tre[:], hs_im[:])
        v.tensor_add(wim[:], t1[:], t2[:])

        # ---- block matrices blkA1 = block(A1), blkA0H = block(A0inv^H)
        def rowcat(tag, left, right, eng=None):
            rt = T([1, 128], tag=tag)
            (eng or v).tensor_copy(rt[0:1, 0:N], left)
            (eng or v).tensor_copy(rt[0:1, N:N2], right)
            return rt

        def blk_from_parts(tag, gre, gim, a_rows, b_rows):
            # block(diag(g)) - block(outer(a,b)); a_rows/b_rows are two
            # [1,128] rows each: block(outer) = sum_r a_rows[r]^T b_rows[r]
            ps = pps.tile([128, 128], F32, tag="pp_ps", name="ps_blk")
            nc.tensor.matmul(ps[:], a_rows[0][:], b_rows[0][:],
                             start=True, stop=False)
            nc.tensor.matmul(ps[:], a_rows[1][:], b_rows[1][:],
                             start=False, stop=True)
            grow = rowcat(tag + "_grow", gre, gre)
            girow = rowcat(tag + "_girow", gim, gim)
            ps2 = pps.tile([128, 2], F32, tag="pp_ps", name="ps_g2")
            nc.tensor.matmul(ps2[:, 0:1], grow[:], one11, start=True, stop=True)
            nc.tensor.matmul(ps2[:, 1:2], girow[:], one11, start=True, stop=True)
            gcol = T([128, 1], tag=tag + "_gcol")
            gicol = T([128, 1], tag=tag + "_gicol")
            v.tensor_copy(gcol[:], ps2[:, 0:1])
            v.tensor_copy(gicol[:], ps2[:, 1:2])
            dg = T([128, 128], tag=tag + "_dg")
            v.tensor_scalar_mul(dg[:], ident, gcol[:])
            dgi = T([128, 128], tag=tag + "_dgi")
            v.tensor_scalar_mul(dgi[:], ilmu, gicol[:])
            out = T([128, 128], tag=tag)
            v.tensor_add(out[:], dg[:], dgi[:])
            v.tensor_sub(out[:], out[:], ps[:])
            return out

        g1re = T([1, N], tag="g1re")
        v.tensor_scalar_add(g1re[:], hlre[:], 1.0)
        a1re = T([1, N], tag="a1re"); v.tensor_scalar_mul(a1re[:], pre, hh[:])
        a1im = T([1, N], tag="a1im"); v.tensor_scalar_mul(a1im[:], pim, hh[:])
        na1im = T([1, N], tag="na1im"); v.tensor_scalar_mul(na1im[:], a1im[:], -1.0)
        a1_rows = [rowcat("a1r0", a1re[:], a1im[:]),
                   rowcat("a1r1", na1im[:], a1re[:])]
        b1_rows = [rowcat("b1r0", pre, pim), rowcat("b1r1", npim[:], pre)]
        blkA1 = blk_from_parts("blkA1", g1re[:], hlim[:], a1_rows, b1_rows)

        nd0im = T([1, N], tag="nd0im"); v.tensor_scalar_mul(nd0im[:], d0im[:], -1.0)
        nvim = T([1, N], tag="nvim"); v.tensor_scalar_mul(nvim[:], vim[:], -1.0)
        nwim = T([1, N], tag="nwim"); v.tensor_scalar_mul(nwim[:], wim[:], -1.0)
        a0_rows = [rowcat("a0r0", vre[:], nvim[:]), rowcat("a0r1", vim[:], vre[:])]
        b0_rows = [rowcat("b0r0", wre[:], wim[:]), rowcat("b0r1", nwim[:], wre[:])]
        blkA0H = blk_from_parts("blkA0H", d0re[:], nd0im[:], a0_rows, b0_rows)

        # ---- helpers -------------------------------------------------
        def ev_copy(eng, dst, src):
            if eng is s:
                s.copy(dst, src)
            else:
                eng.tensor_copy(dst, src)

        def mm_ev(lhsT, rhs, m, n_, tag, eng=None):
            ps = pps.tile([128, max(n_, 1)], F32, tag="pp_ps", name="ps_mm")
            nc.tensor.matmul(ps[0:m, 0:n_], lhsT[:], rhs[:], start=True, stop=True)
            t = T([m, n_], tag=tag)
            ev_copy(eng or v, t[:], ps[0:m, 0:n_])
            return t

        cast = con.tile

        def bf(name, srcf, eng=None):
            t = cast([128, 128], BF16, tag=name, name=name)
            ev_copy(eng or v, t[:], srcf[:])
            return t

        # ---- Bbar: b2 = block(A0inv) @ (delta*B)  (V chain needs it) -
        psb = pps.tile([128, 1], F32, tag="pp_ps", name="ps_bcol")
        nc.tensor.matmul(psb[:, 0:1], brow_s[:], one11, start=True, stop=True)
        bcol = T([128, 1], tag="bcol")
        v.tensor_copy(bcol[:], psb[:, 0:1])
        b2 = mm_ev(blkA0H, bcol, 128, 1, "b2")

        # ---- Abar pair + squaring chain ------------------------------
        NSQ = 12   # Abar^(2^12) = Abar^4096
        A2 = [None] * (NSQ + 1)
        A2T = [None] * (NSQ + 1)
        A2[0] = mm_ev(blkA0H, blkA1, 128, 128, "A2_0")
        A2T[0] = mm_ev(blkA1, blkA0H, 128, 128, "A2T_0", eng=s)

        def sq_pair(k, need_at=True, bf16=False):
            dt_out = BF16 if bf16 else F32
            ps = pps.tile([128, 256], F32, tag="pp_ps", name="sqpair")
            nc.tensor.matmul(ps[:, 0:128], A2T[k][:], A2[k][:],
                             start=True, stop=(not need_at))
            if need_at:
                nc.tensor.matmul(ps[:, 128:256], A2[k][:], A2T[k][:],
                                 start=False, stop=True)
            a = T([128, 128], dt=dt_out, tag=f"A2_{k+1}")
            v.tensor_copy(a[:], ps[:, 0:128])
            at = None
            if need_at:
                at = T([128, 128], dt=dt_out, tag=f"A2T_{k+1}")
                s.copy(at[:], ps[:, 128:256])
            return a, at

        # Ccol chain and V chain interleave with the squaring chain; the
        # dataflow scheduler runs them in the chain's gaps.
        c0row = rowcat("c0row", cre, cimr)
        c1row = rowcat("c1row", ncim[:], cre)
        Ccol = T([128, 128], tag="Ccol")
        psc = pps.tile([128, 128], F32, tag="pp_ps", name="psc")
        nc.tensor.matmul(psc[:, 0:1], c0row[:], one11, start=True, stop=False)
        nc.tensor.matmul(psc[:, 1:2], c1row[:], one11, start=False, stop=True)
        v.tensor_copy(Ccol[:, 0:2], psc[:, 0:2])

        V = T([128, 128], tag="Vd")
        psv0 = pps.tile([128, 128], F32, tag="pp_ps", name="psv0")
        nc.tensor.matmul(psv0[:, 0:1], A2T[0][:], b2[:], start=True, stop=True)
        s.copy(V[:, 0:1], psv0[:, 0:1])

        def ccol_step(k):
            nr = 2 << k
            psr = pps.tile([128, 128], F32, tag="pp_ps", name="psr")
            nc.tensor.matmul(psr[:, 0:nr], A2[k][:], Ccol[:, 0:nr],
                             start=True, stop=True)
            v.tensor_copy(Ccol[:, nr:2 * nr], psr[:, 0:nr])

        def v_step(k):
            wd = 1 << k
            psv = pps.tile([128, 128], F32, tag="pp_ps", name="psv")
            nc.tensor.matmul(psv[:, 0:wd], A2T[k][:], V[:, 0:wd],
                             start=True, stop=True)
            s.copy(V[:, wd:2 * wd], psv[:, 0:wd])

        for k in range(7):
            A2[k + 1], A2T[k + 1] = sq_pair(k)
            if k > 0:
                v_step(k - 1)
                ccol_step(k - 1)

        v_step(5); v_step(6)
        Ccol_hi = mm_ev(A2[6], Ccol, 128, 128, "Ccol_hi")
        Wout_f = T([128, 128], tag="Wout_f")
        v.tensor_copy(Wout_f[:, 0:64], Ccol[:, 0:128:2])
        v.tensor_copy(Wout_f[:, 64:128], Ccol_hi[:, 0:128:2])

        # W = V^T via PE; Min = rev @ W ; MinT = W^T rev = V @ rev
        psw = pps.tile([128, 128], F32, tag="pp_ps", name="psw")
        nc.tensor.transpose(psw[:], V[:], ident)
        W = T([128, 128], tag="Wd")
        v.tensor_copy(W[:], psw[:])
        Min_f = mm_ev(revm, W, 128, 128, "Min_f", eng=s)
        MinT_f = mm_ev(W, revm, 128, 128, "MinT_f")
        E_f = mm_ev(MinT_f, A2T[7], 128, 128, "E_f", eng=s)

        Min_bf = bf("Min_bf", Min_f)
        E_bf = bf("E_bf", E_f, eng=s)

        # late squarings (bf16): cast the k=7 pair, then chain in bf16
        A2_7_f32, A2T_7_f32 = A2[7], A2T[7]    # fp32 A^128 kept for W1/E
        if BF16_CHAIN:
            A2[7] = bf("A2b7", A2[7])          # bf16 shadows for the chain
            A2T[7] = bf("A2Tb7", A2T[7], eng=s)
            for k in range(7, NSQ):
                A2[k + 1], A2T[k + 1] = sq_pair(k, need_at=(k < NSQ - 1),
                                                bf16=True)
            Dq2T_bf = A2T[8]                   # already bf16
        else:
            for k in range(7, NSQ):
                A2[k + 1], A2T[k + 1] = sq_pair(k, need_at=(k < NSQ - 1))
            Dq2T_bf = bf("Dq2T_bf", A2T[8])

        # ================= main loop setup ============================
        hp = self.pool("h", 3)
        yp = self.pool("yt", 3)
        ph_p = self.pool("ph", 2, "PSUM")
        py_p = self.pool("py", 2, "PSUM")
        yt_r = yt_d.rearrange("i t b -> t i b")

        h_tiles = [None] * NPAIR

        def trio(k):
            # h_k = Dq2 h_{k-1} + E u_{2k-2} + Min u_{2k-1}
            ph = ph_p.tile([128, BC], F32, tag="ph", name="ph")
            first = True
            if h_tiles[k - 1] is not None:
                nc.tensor.matmul(ph[:], Dq2T_bf[:], h_tiles[k - 1][:],
                                 start=True, stop=False)
                first = False
            nc.tensor.matmul(ph[:], E_bf[:], u_of(2 * k - 2),
                             start=first, stop=False)
            nc.tensor.matmul(ph[:], Min_bf[:], u_of(2 * k - 1),
                             start=False, stop=True)
            h_cur = hp.tile([128, BC], BF16, tag="h", name="h")
            ev_copy(v if k % 2 else s, h_cur[:], ph[:])
            h_tiles[k] = h_cur

        def consumer(j):
            # both chunks of pair j in one 2-bank PSUM tile; T0 terms
            # first so the near field runs before h_j lands (in-bank
            # accumulation executes in program order)
            h_j = h_tiles[j]
            py = py_p.tile([128, 2 * BC], F32, tag="py", name="py")
            if h_j is not None:
                nc.tensor.matmul(py[:, 0:BC], T0_bf[:], u_of(2 * j),
                                 start=True, stop=False)
                nc.tensor.matmul(py[:, BC:2 * BC], T0_bf[:], u_of(2 * j + 1),
                                 start=True, stop=False)
                nc.tensor.matmul(py[:, BC:2 * BC], G0_bf[:], u_of(2 * j),
                                 start=False, stop=False)
                nc.tensor.matmul(py[:, 0:BC], Wout_bf[:], h_j[:],
                                 start=False, stop=True)
                nc.tensor.matmul(py[:, BC:2 * BC], W1_bf[:], h_j[:],
                                 start=False, stop=True)
            else:
                nc.tensor.matmul(py[:, 0:BC], T0_bf[:], u_of(2 * j),
                                 start=True, stop=True)
                nc.tensor.matmul(py[:, BC:2 * BC], T0_bf[:], u_of(2 * j + 1),
                                 start=True, stop=False)
                nc.tensor.matmul(py[:, BC:2 * BC], G0_bf[:], u_of(2 * j),
                                 start=False, stop=True)
            yt_t = yp.tile([128, 2, BC], BF16, tag="ytt", name="ytt")
            ev_copy(s if j % 2 else v, yt_t[:], py[:])
            nc.sync.dma_start(out=yt_r[:, 2 * j:2 * j + 2, :], in_=yt_t[:])

        # trios can pre-run as soon as Min/E/Dq2 are ready
        trio(1); trio(2)

        # ---- alias correction on the C side: Wout'' = (I+X^T+X^2T)Wout
        if BF16_CHAIN:
            Wout_pre_bf = bf("Wout_pre_bf", Wout_f)
            wc1 = mm_ev(A2[NSQ], Wout_pre_bf, 128, 128, "wc1")
            trio(3)
            wc1b = bf("wc1b", wc1)
            wc2 = mm_ev(A2[NSQ], wc1b, 128, 128, "wc2", eng=s)
        else:
            wc1 = mm_ev(A2[NSQ], Wout_f, 128, 128, "wc1")
            trio(3)
            wc2 = mm_ev(A2[NSQ], wc1, 128, 128, "wc2", eng=s)
        v.tensor_add(Wout_f[:], Wout_f[:], wc1[:])
        v.tensor_add(Wout_f[:], Wout_f[:], wc2[:])
        Wout_bf = bf("Wout_bf", Wout_f)
        trio(4)

        # ---- K taps row -> Toeplitz T0 via DRAM shift trick ----------
        psk = pps.tile([128, 128], F32, tag="pp_ps", name="psk")
        nc.tensor.matmul(psk[0:1, 0:128], b2[:], Wout_f[:], start=True, stop=True)
        zK = T([1, 128], tag="zK")
        v.tensor_copy(zK[:], psk[0:1, 0:128])
        v.tensor_add(zK[0:1, 0:1], zK[0:1, 0:1], dval)   # += D at lag 0
        nc.sync.dma_start(out=zs[128:256], in_=zK[:])
        trio(5)
        # T0R[p, t] = zs[1 + p + t] = T0[127-p, t]; un-reverse via rev@T0R
        T0R = T([128, 128], tag="T0R")
        zsap = zs[:]
        src = bass.AP(zsap.tensor, zsap.offset + 1, [[1, 128], [1, 128]])
        nc.sync.dma_start(out=T0R[:], in_=src)
        # remaining far maps while the DMAs fly:
        # W1 = block(Dq)^T @ Wout'' ; G0 = Min @ Wout''
        W1_f = mm_ev(A2_7_f32, Wout_f, 128, 128, "W1_f")
        G0_f = mm_ev(MinT_f, Wout_f, 128, 128, "G0_f", eng=s)
        W1_bf = bf("W1_bf", W1_f)
        G0_bf = bf("G0_bf", G0_f, eng=s)
        trio(6)
        T0f = mm_ev(revm, T0R, 128, 128, "T0f")
        T0_bf = bf("T0_bf", T0f)
        trio(7)

        # ================= main loop ==================================
        nxt = 0
        for it in range(8, NPAIR):
            trio(it)
            consumer(nxt); nxt += 1
        while nxt < NPAIR:
            consumer(nxt); nxt += 1


def kernel(**inputs):
    global LAST_EXEC_NS, LAST_RESULTS
    nc = build_program()
    cmat = _consts()

    u = np.asarray(inputs["u"], dtype=np.float32)
    # per-core pre-transpose to [q, i, (j, b')] and cast to bf16
    # row = c*512 + j*128 + b', col = i*128 + q
    ut = u.reshape(NCORES, 4, 128, NCH, 128).transpose(0, 4, 3, 1, 2)
    ut = np.ascontiguousarray(ut).reshape(NCORES, 128, NCH * BC)
    ut = ut.astype(ml_dtypes.bfloat16)

    par = np.zeros((1, PARW), dtype=np.float32)
    par[0, O_LRE:O_LRE + N] = np.asarray(inputs["Lambda_re"], np.float32)
    par[0, O_LIM:O_LIM + N] = np.asarray(inputs["Lambda_im"], np.float32)
    par[0, O_PRE:O_PRE + N] = np.asarray(inputs["P_re"], np.float32)
    par[0, O_PIM:O_PIM + N] = np.asarray(inputs["P_im"], np.float32)
    par[0, O_BRE:O_BRE + N] = np.asarray(inputs["B_re"], np.float32)
    par[0, O_BIM:O_BIM + N] = np.asarray(inputs["B_im"], np.float32)
    par[0, O_CRE:O_CRE + N] = np.asarray(inputs["C_ri"], np.float32)[:, 0]
    par[0, O_CIM:O_CIM + N] = np.asarray(inputs["C_ri"], np.float32)[:, 1]
    par[0, O_D] = np.asarray(inputs["D"], np.float32).reshape(-1)[0]
    par[0, O_LSTEP] = np.asarray(inputs["log_step"], np.float32).reshape(-1)[0]
    par[0, O_ONE] = 1.0

    in_maps = []
    for c in range(NCORES):
        in_maps.append({"ut": ut[c], "par": par, "cmat": cmat})

    trace = bool(int(os.environ.get("KERNEL_TRACE", "0")))
    kw = {}
    if trace:
        kw["trace"] = True
        kw["trace_cores"] = list(range(NCORES))
    res = run_bass_kernel_spmd(nc, in_maps, list(range(NCORES)), **kw)
    LAST_EXEC_NS = res.exec_time_ns
    LAST_RESULTS = res

    y = np.empty((BH, L), dtype=np.float32)
    for c in range(NCORES):
        ytc = np.asarray(res.results[c]["yt"]).astype(np.float32)  # [i, t, jb]
        y[c * BC:(c + 1) * BC] = (
            ytc.reshape(NCH, 128, 4, 128).transpose(2, 3, 0, 1).reshape(BC, L))
    return y


# revision 48
# speedup vs baseline: 1.0867x; 1.0474x over previous
"""Trainium2 Bass kernel for nn_LiquidS4Layer (S4 DPLR forward).

y = causal_conv(u, K) + D*u, with K the length-L SSM kernel computed from
small DPLR params (Lambda, P, B, C, step).

Algorithm (all on device, per core over 512 of the 4096 batch rows):
  1. Discretize via bilinear transform using the Woodbury identity
     (A = Lambda - P P^H is diagonal + rank-1, so (I - h A)^-1 is closed
     form): build block-real forms of Abar, Bbar.
  2. Alias-correct: the reference kernel is the *periodized* kernel
     K[l] = sum_m K_inf[l + m L]; equivalently apply (I - Abar^L)^-1
     (truncated Neumann series, Abar^L from repeated squaring).  The
     correction is applied on the C side (Wout'' = (I + X^T + X^2T) Wout,
     X = Abar^L, commutes with powers of Abar) so the B-side chains
     (V doubling -> Min/E) depend only on early squarings and overlap
     the squaring chain.
  3. Chunked convolution (chunk Q=128): per chunk, intra-chunk causal
     Toeplitz matmul with K[0:Q] (+ D on the diagonal), plus a rank-2N
     state passing: states h_i = Abar^Q h_{i-1} + Proj(u chunk i-1),
     far-field y += Re(Wout h_i).

Main loop is weight-stationary: the small [128,128] operator matrices
(T0 Toeplitz, Wout/W1 far-field maps, G0 direct map, Dq2T/E/Min state
maps) are the PE's stationary lhsT; u chunks and h stream through as
512-wide moving operands.  The h-update recurrence (trios) runs pairs
ahead of the near/far consumers.  The late squarings (k>=7) run in bf16
(their products only feed bf16 main-loop operators or the <=15% alias
correction).  Output is produced transposed ([t, batch]) and
un-transposed on the host.  u arrives pre-transposed/bf16 from the
host; y leaves as bf16.  Params arrive packed in one fp32 row.

Sharding: u/(y) row-sharded over 8 cores (batch*channel parallel); the
small params are replicated; no collectives.
"""
import os
import numpy as np
from contextlib import ExitStack

import ml_dtypes

import concourse.bass as bass
import concourse.tile as tile
from concourse import mybir
from concourse.bass_utils import run_bass_kernel_spmd

F32 = mybir.dt.float32
BF16 = mybir.dt.bfloat16

NCORES = 8
BH, L = 4096, 4096
BC = BH // NCORES       # 512 rows per core
N = 64                  # SSM state size
N2 = 2 * N              # real block state size = 128
Q = 128                 # chunk length
NCH = L // Q            # 32 chunks
NPAIR = NCH // 2

BF16_CHAIN = False      # late squarings (k>=7) in bf16

# packed param row offsets (par tensor, [1, PARW] fp32)
O_LRE, O_LIM, O_PRE, O_PIM = 0, 64, 128, 192
O_BRE, O_BIM, O_CRE, O_CIM = 256, 320, 384, 448
O_D, O_LSTEP, O_ONE = 512, 513, 514
PARW = 640

LAST_EXEC_NS = None
LAST_RESULTS = None


def _consts():
    ident = np.eye(128, dtype=np.float32)
    rev = ident[::-1].copy()                      # antidiagonal reversal
    ilmu = np.zeros((128, 128), dtype=np.float32)  # IL - IU blocks
    for p in range(64):
        ilmu[p, p + 64] = -1.0                    # -IU (top-right)
        ilmu[p + 64, p] = 1.0                     # +IL (bottom-left)
    return np.concatenate([ident, rev, ilmu], axis=1)  # [128, 384]


def build_program():
    nc = bass.Bass()
    dp = nc.declare_dram_parameter
    ut = dp("ut", [128, NCH * BC], BF16, isOutput=False)   # [q, (i, j, b')]
    yt = dp("yt", [NCH, 128, BC], BF16, isOutput=True)     # [i, t, (j, b')]
    par = dp("par", [1, PARW], F32, isOutput=False)
    cmat = dp("cmat", [128, 384], F32, isOutput=False)

    with TileKernel(nc) as tk:
        tk.build(ut, yt, par, cmat)
    _split_multi_waits(nc)
    return nc


def _split_multi_waits(nc):
    """This toolchain's walrus encodes at most one sync wait per (non-Drain)
    instruction.  Tile can emit several; hoist the extras onto standalone
    EventSemaphore wait instructions inserted just before, on the same
    engine (engines execute their stream in order, so this is equivalent)."""
    ctr = 0
    for f in nc.m.functions:
        for blk in f.blocks:
            out = []
            changed = False
            for inst in blk.instructions:
                si = inst.sync_info
                if si is None:
                    out.append(inst)
                    continue
                waits = list(si.on_wait)
                if len(waits) > 1:
                    # pick a non-DMA sem for the no-op update (the sim
                    # forbids foreign updates of in-flight DMA sems)
                    cands = [u for u in si.on_update] + [
                        w for w in waits if "DMA" not in w.ant_name]
                    for w in waits[:-1]:
                        ev = mybir.InstEventSemaphore(
                            name=f"I-wsplit-{ctr}", ins=[], outs=[])
                        ctr += 1
                        ev.engine = inst.engine
                        # zero-increment update: the sim requires >=1 update
                        # per instruction; +0 changes no semaphore value.
                        c = cands[0] if cands else w
                        up = mybir.SyncUpdate(
                            sync_type="semaphore", id=c.id, ant_name=c.ant_name,
                            update_mode="sem-add-imm", update_value=0,
                            update_reg=None)
                        ev.sync_info = mybir.SyncInfo(on_wait=[w], on_update=[up])
                        out.append(ev)
                    inst.sync_info = mybir.SyncInfo(
                        on_wait=[waits[-1]], on_update=list(si.on_update))
                    changed = True
                out.append(inst)
            if changed:
                blk.instructions = out


class TileKernel:
    def __init__(self, nc):
        self.nc = nc
        self.ctx = ExitStack()
        self.tc = tile.TileContext(nc)

    def __enter__(self):
        self.ctx.__enter__()
        self.tc.__enter__()
        return self

    def __exit__(self, *a):
        self.ctx.__exit__(*a)   # release pools before the scheduler runs
        return self.tc.__exit__(*a)

    def pool(self, name, bufs=1, space="SBUF"):
        return self.ctx.enter_context(
            self.tc.tile_pool(name=name, bufs=bufs, space=space))

    def build(self, ut_d, yt_d, par_d, cmat_d):
        nc, tc = self.nc, self.tc
        con = self.pool("const", 1)
        pp = self.pool("pp", 1)          # param pipeline tiles (unique tags)
        pps = self.pool("pps", 2, "PSUM")
        dram = self.pool("dram", 1, "DRAM")

        def T(shape, dt=F32, p=pp, tag=None):
            return p.tile(shape, dt, tag=tag, name=tag)

        v = nc.vector
        s = nc.scalar
        g = nc.gpsimd

        # ---- load packed params & constants (2 DMAs) and u (4 DMAs) ---
        par = con.tile([1, PARW], F32, tag="par", name="par")
        nc.sync.dma_start(out=par[:], in_=par_d[:])
        cmt = con.tile([128, 384], F32, tag="cmt", name="cmt")
        nc.scalar.dma_start(out=cmt[:], in_=cmat_d[:])

        ut_sb = con.tile([128, NCH * BC], BF16, tag="ut", name="ut")
        NPC = 4  # u load pieces
        W_PC = NCH * BC // NPC
        for pc in range(NPC):
            nc.gpsimd.dma_start(out=ut_sb[:, pc * W_PC:(pc + 1) * W_PC],
                                in_=ut_d[:, pc * W_PC:(pc + 1) * W_PC])

        def u_of(i):
            return ut_sb[:, i * BC:(i + 1) * BC]

        # zero prefix of the Toeplitz scratch (independent -> issue early)
        zs = dram.tile([256], F32, tag="zscratch", name="zscratch")
        zrow = T([1, 128], tag="zrow")
        v.memset(zrow[:], 0.0)
        nc.gpsimd.dma_start(out=zs[0:128], in_=zrow[:])

        lre = par[0:1, O_LRE:O_LRE + N]
        lim = par[0:1, O_LIM:O_LIM + N]
        pre = par[0:1, O_PRE:O_PRE + N]
        pim = par[0:1, O_PIM:O_PIM + N]
        bre = par[0:1, O_BRE:O_BRE + N]
        bim = par[0:1, O_BIM:O_BIM + N]
        cre = par[0:1, O_CRE:O_CRE + N]
        cimr = par[0:1, O_CIM:O_CIM + N]
        dval = par[0:1, O_D:O_D + 1]
        lstep = par[0:1, O_LSTEP:O_LSTEP + 1]
        one11 = par[0:1, O_ONE:O_ONE + 1]
        ident = cmt[:, 0:128]
        revm = cmt[:, 128:256]
        ilmu = cmt[:, 256:384]

        # ---- tiny param pipeline: spine on vector, side work on gpsimd
        # (gpsimd is SBUF-only, which all of these are)
        delta = T([1, 1], tag="delta")
        s.activation(delta[:], lstep, mybir.ActivationFunctionType.Exp)
        hh = T([1, 1], tag="hh")
        v.tensor_scalar_mul(hh[:], delta[:], 0.5)

        # independent-of-delta side work (gpsimd, starts as soon as par lands)
        p2 = T([1, N], tag="p2"); g1 = T([1, N], tag="g1"); g2 = T([1, N], tag="g2")
        v.tensor_mul(g1[:], pre, pre); v.tensor_mul(g2[:], pim, pim)
        v.tensor_add(p2[:], g1[:], g2[:])
        npim = T([1, N], tag="npim"); v.tensor_scalar_mul(npim[:], pim, -1.0)
        ncim = T([1, N], tag="ncim"); v.tensor_scalar_mul(ncim[:], cimr, -1.0)
        brow = T([1, 128], tag="brow")
        v.tensor_copy(brow[0:1, 0:N], bre)
        v.tensor_copy(brow[0:1, N:N2], bim)

        hlre = T([1, N], tag="hlre"); v.tensor_scalar_mul(hlre[:], lre, hh[:])
        hlim = T([1, N], tag="hlim"); v.tensor_scalar_mul(hlim[:], lim, hh[:])
        den_re = T([1, N], tag="den_re")
        v.tensor_scalar(den_re[:], hlre[:], -1.0, 1.0,
                        op0=mybir.AluOpType.mult, op1=mybir.AluOpType.add)
        den_im = T([1, N], tag="den_im")
        v.tensor_scalar_mul(den_im[:], hlim[:], -1.0)
        brow_s = T([1, 128], tag="brow_s")
        v.tensor_scalar_mul(brow_s[:], brow[:], delta[:])

        t1 = T([1, N], tag="t1"); t2 = T([1, N], tag="t2")
        r2 = T([1, N], tag="r2")
        v.tensor_mul(t1[:], den_re[:], den_re[:])
        v.tensor_mul(t2[:], den_im[:], den_im[:])
        v.tensor_add(r2[:], t1[:], t2[:])
        rinv = T([1, N], tag="rinv"); v.reciprocal(rinv[:], r2[:])
        d0re = T([1, N], tag="d0re"); v.tensor_mul(d0re[:], den_re[:], rinv[:])
        nden_im = T([1, N], tag="nden_im")
        v.tensor_scalar_mul(nden_im[:], den_im[:], -1.0)
        d0im = T([1, N], tag="d0im"); v.tensor_mul(d0im[:], nden_im[:], rinv[:])

        # s = 1 + h * sum(|P|^2 d0)  (complex)
        sr = T([1, 1], tag="sr"); si = T([1, 1], tag="si")
        v.tensor_mul(t1[:], p2[:], d0re[:])
        v.reduce_sum(sr[:], t1[:], axis=mybir.AxisListType.X)
        v.tensor_mul(t2[:], p2[:], d0im[:])
        v.reduce_sum(si[:], t2[:], axis=mybir.AxisListType.X)
        s_re = T([1, 1], tag="s_re")
        v.tensor_scalar(s_re[:], sr[:], hh[:], 1.0,
                        op0=mybir.AluOpType.mult, op1=mybir.AluOpType.add)
        s_im = T([1, 1], tag="s_im"); v.tensor_mul(s_im[:], si[:], hh[:])
        # hs = h / s  (complex)
        s2 = T([1, 1], tag="s2"); sa = T([1, 1], tag="sa"); sb = T([1, 1], tag="sb")
        v.tensor_mul(sa[:], s_re[:], s_re[:]); v.tensor_mul(sb[:], s_im[:], s_im[:])
        v.tensor_add(s2[:], sa[:], sb[:])
        s2i = T([1, 1], tag="s2i"); v.reciprocal(s2i[:], s2[:])
        hs_re = T([1, 1], tag="hs_re"); hs_im = T([1, 1], tag="hs_im")
        v.tensor_mul(sa[:], s_re[:], s2i[:]); v.tensor_mul(hs_re[:], sa[:], hh[:])
        v.tensor_mul(sb[:], s_im[:], s2i[:]); v.tensor_mul(sa[:], sb[:], hh[:])
        v.tensor_scalar_mul(hs_im[:], sa[:], -1.0)

        # t = d0*P ; w = hs * t ; vv = conj(P)*d0
        tre = T([1, N], tag="tre"); tim = T([1, N], tag="tim")
        v.tensor_mul(t1[:], d0re[:], pre); v.tensor_mul(t2[:], d0im[:], pim)
        v.tensor_sub(tre[:], t1[:], t2[:])
        v.tensor_mul(t1[:], d0re[:], pim); v.tensor_mul(t2[:], d0im[:], pre)
        v.tensor_add(tim[:], t1[:], t2[:])
        vre = T([1, N], tag="vre"); vim = T([1, N], tag="vim")
        v.tensor_mul(t1[:], pre, d0re[:]); v.tensor_mul(t2[:], pim, d0im[:])
        v.tensor_add(vre[:], t1[:], t2[:])
        v.tensor_mul(t1[:], pre, d0im[:]); v.tensor_mul(t2[:], pim, d0re[:])
        v.tensor_sub(vim[:], t1[:], t2[:])
        wre = T([1, N], tag="wre"); wim = T([1, N], tag="wim")
        v.tensor_scalar_mul(t1[:], tre[:], hs_re[:])
        v.tensor_scalar_mul(t2[:], tim[:], hs_im[:])
        v.tensor_sub(wre[:], t1[:], t2[:])
        v.tensor_scalar_mul(t1[:], tim[:], hs_re[:])
        v.tensor_scalar_mul(t2[:], tre[:], hs_im[:])
        v.tensor_add(wim[:], t1[:], t2[:])

        # ---- block matrices blkA1 = block(A1), blkA0H = block(A0inv^H)
        def rowcat(tag, left, right, eng=None):
            rt = T([1, 128], tag=tag)
            (eng or v).tensor_copy(rt[0:1, 0:N], left)
            (eng or v).tensor_copy(rt[0:1, N:N2], right)
            return rt

        def blk_from_parts(tag, gre, gim, a_rows, b_rows):
            # block(diag(g)) - block(outer(a,b)); a_rows/b_rows are two
            # [1,128] rows each: block(outer) = sum_r a_rows[r]^T b_rows[r]
            ps = pps.tile([128, 128], F32, tag="pp_ps", name="ps_blk")
            nc.tensor.matmul(ps[:], a_rows[0][:], b_rows[0][:],
                             start=True, stop=False)
            nc.tensor.matmul(ps[:], a_rows[1][:], b_rows[1][:],
                             start=False, stop=True)
            grow = rowcat(tag + "_grow", gre, gre)
            girow = rowcat(tag + "_girow", gim, gim)
            ps2 = pps.tile([128, 2], F32, tag="pp_ps", name="ps_g2")
            nc.tensor.matmul(ps2[:, 0:1], grow[:], one11, start=True, stop=True)
            nc.tensor.matmul(ps2[:, 1:2], girow[:], one11, start=True, stop=True)
            gcol = T([128, 1], tag=tag + "_gcol")
            gicol = T([128, 1], tag=tag + "_gicol")
            v.tensor_copy(gcol[:], ps2[:, 0:1])
            v.tensor_copy(gicol[:], ps2[:, 1:2])
            dg = T([128, 128], tag=tag + "_dg")
            v.tensor_scalar_mul(dg[:], ident, gcol[:])
            dgi = T([128, 128], tag=tag + "_dgi")
            v.tensor_scalar_mul(dgi[:], ilmu, gicol[:])
            out = T([128, 128], tag=tag)
            v.tensor_add(out[:], dg[:], dgi[:])
            v.tensor_sub(out[:], out[:], ps[:])
            return out

        g1re = T([1, N], tag="g1re")
        v.tensor_scalar_add(g1re[:], hlre[:], 1.0)
        a1re = T([1, N], tag="a1re"); v.tensor_scalar_mul(a1re[:], pre, hh[:])
        a1im = T([1, N], tag="a1im"); v.tensor_scalar_mul(a1im[:], pim, hh[:])
        na1im = T([1, N], tag="na1im"); v.tensor_scalar_mul(na1im[:], a1im[:], -1.0)
        a1_rows = [rowcat("a1r0", a1re[:], a1im[:]),
                   rowcat("a1r1", na1im[:], a1re[:])]
        b1_rows = [rowcat("b1r0", pre, pim), rowcat("b1r1", npim[:], pre)]
        blkA1 = blk_from_parts("blkA1", g1re[:], hlim[:], a1_rows, b1_rows)

        nd0im = T([1, N], tag="nd0im"); v.tensor_scalar_mul(nd0im[:], d0im[:], -1.0)
        nvim = T([1, N], tag="nvim"); v.tensor_scalar_mul(nvim[:], vim[:], -1.0)
        nwim = T([1, N], tag="nwim"); v.tensor_scalar_mul(nwim[:], wim[:], -1.0)
        a0_rows = [rowcat("a0r0", vre[:], nvim[:]), rowcat("a0r1", vim[:], vre[:])]
        b0_rows = [rowcat("b0r0", wre[:], wim[:]), rowcat("b0r1", nwim[:], wre[:])]
        blkA0H = blk_from_parts("blkA0H", d0re[:], nd0im[:], a0_rows, b0_rows)

        # ---- helpers -------------------------------------------------
        def ev_copy(eng, dst, src):
            if eng is s:
                s.copy(dst, src)
            else:
                eng.tensor_copy(dst, src)

        def mm_ev(lhsT, rhs, m, n_, tag, eng=None):
            ps = pps.tile([128, max(n_, 1)], F32, tag="pp_ps", name="ps_mm")
            nc.tensor.matmul(ps[0:m, 0:n_], lhsT[:], rhs[:], start=True, stop=True)
            t = T([m, n_], tag=tag)
            ev_copy(eng or v, t[:], ps[0:m, 0:n_])
            return t

        cast = con.tile

        def bf(name, srcf, eng=None):
            t = cast([128, 128], BF16, tag=name, name=name)
            ev_copy(eng or v, t[:], srcf[:])
            return t

        # ---- Bbar: b2 = block(A0inv) @ (delta*B)  (V chain needs it) -
        psb = pps.tile([128, 1], F32, tag="pp_ps", name="ps_bcol")
        nc.tensor.matmul(psb[:, 0:1], brow_s[:], one11, start=True, stop=True)
        bcol = T([128, 1], tag="bcol")
        v.tensor_copy(bcol[:], psb[:, 0:1])
        b2 = mm_ev(blkA0H, bcol, 128, 1, "b2")

        # ---- Abar pair + squaring chain ------------------------------
        NSQ = 12   # Abar^(2^12) = Abar^4096
        A2 = [None] * (NSQ + 1)
        A2T = [None] * (NSQ + 1)
        A2[0] = mm_ev(blkA0H, blkA1, 128, 128, "A2_0")
        A2T[0] = mm_ev(blkA1, blkA0H, 128, 128, "A2T_0", eng=s)

        def sq_pair(k, need_at=True, bf16=False):
            dt_out = BF16 if bf16 else F32
            ps = pps.tile([128, 256], F32, tag="pp_ps", name="sqpair")
            nc.tensor.matmul(ps[:, 0:128], A2T[k][:], A2[k][:],
                             start=True, stop=(not need_at))
            if need_at:
                nc.tensor.matmul(ps[:, 128:256], A2[k][:], A2T[k][:],
                                 start=False, stop=True)
            a = T([128, 128], dt=dt_out, tag=f"A2_{k+1}")
            v.tensor_copy(a[:], ps[:, 0:128])
            at = None
            if need_at:
                at = T([128, 128], dt=dt_out, tag=f"A2T_{k+1}")
                s.copy(at[:], ps[:, 128:256])
            return a, at

        # Ccol chain and V chain interleave with the squaring chain; the
        # dataflow scheduler runs them in the chain's gaps.
        c0row = rowcat("c0row", cre, cimr)
        c1row = rowcat("c1row", ncim[:], cre)
        Ccol = T([128, 128], tag="Ccol")
        psc = pps.tile([128, 128], F32, tag="pp_ps", name="psc")
        nc.tensor.matmul(psc[:, 0:1], c0row[:], one11, start=True, stop=False)
        nc.tensor.matmul(psc[:, 1:2], c1row[:], one11, start=False, stop=True)
        v.tensor_copy(Ccol[:, 0:2], psc[:, 0:2])

        V = T([128, 128], tag="Vd")
        psv0 = pps.tile([128, 128], F32, tag="pp_ps", name="psv0")
        nc.tensor.matmul(psv0[:, 0:1], A2T[0][:], b2[:], start=True, stop=True)
        s.copy(V[:, 0:1], psv0[:, 0:1])

        def ccol_step(k):
            nr = 2 << k
            psr = pps.tile([128, 128], F32, tag="pp_ps", name="psr")
            nc.tensor.matmul(psr[:, 0:nr], A2[k][:], Ccol[:, 0:nr],
                             start=True, stop=True)
            v.tensor_copy(Ccol[:, nr:2 * nr], psr[:, 0:nr])

        def v_step(k):
            wd = 1 << k
            psv = pps.tile([128, 128], F32, tag="pp_ps", name="psv")
            nc.tensor.matmul(psv[:, 0:wd], A2T[k][:], V[:, 0:wd],
                             start=True, stop=True)
            s.copy(V[:, wd:2 * wd], psv[:, 0:wd])

        for k in range(7):
            A2[k + 1], A2T[k + 1] = sq_pair(k)
            if k > 0:
                v_step(k - 1)
                ccol_step(k - 1)

        v_step(6)
        Ccol_hi = mm_ev(A2[6], Ccol, 128, 128, "Ccol_hi")
        Wout_f = T([128, 128], tag="Wout_f")
        v.tensor_copy(Wout_f[:, 0:64], Ccol[:, 0:128:2])
        v.tensor_copy(Wout_f[:, 64:128], Ccol_hi[:, 0:128:2])

        # W = V^T via PE; Min = rev @ W ; MinT = W^T rev = V @ rev
        psw = pps.tile([128, 128], F32, tag="pp_ps", name="psw")
        nc.tensor.transpose(psw[:], V[:], ident)
        W = T([128, 128], tag="Wd")
        v.tensor_copy(W[:], psw[:])
        Min_f = mm_ev(revm, W, 128, 128, "Min_f", eng=s)
        MinT_f = mm_ev(W, revm, 128, 128, "MinT_f")
        E_f = mm_ev(MinT_f, A2T[7], 128, 128, "E_f", eng=s)

        Min_bf = bf("Min_bf", Min_f)
        E_bf = bf("E_bf", E_f, eng=s)

        # late squarings (bf16): cast the k=7 pair, then chain in bf16
        A2_7_f32, A2T_7_f32 = A2[7], A2T[7]    # fp32 A^128 kept for W1/E
        if BF16_CHAIN:
            A2[7] = bf("A2b7", A2[7])          # bf16 shadows for the chain
            A2T[7] = bf("A2Tb7", A2T[7], eng=s)
            for k in range(7, NSQ):
                A2[k + 1], A2T[k + 1] = sq_pair(k, need_at=(k < NSQ - 1),
                                                bf16=True)
            Dq2T_bf = A2T[8]                   # already bf16
        else:
            for k in range(7, NSQ):
                A2[k + 1], A2T[k + 1] = sq_pair(k, need_at=(k < NSQ - 1))
            Dq2T_bf = bf("Dq2T_bf", A2T[8])

        # ================= main loop setup ============================
        hp = self.pool("h", 3)
        yp = self.pool("yt", 3)
        ph_p = self.pool("ph", 2, "PSUM")
        py_p = self.pool("py", 2, "PSUM")
        yt_r = yt_d.rearrange("i t b -> t i b")

        h_tiles = [None] * NPAIR

        def trio(k):
            # h_k = Dq2 h_{k-1} + E u_{2k-2} + Min u_{2k-1}
            ph = ph_p.tile([128, BC], F32, tag="ph", name="ph")
            first = True
            if h_tiles[k - 1] is not None:
                nc.tensor.matmul(ph[:], Dq2T_bf[:], h_tiles[k - 1][:],
                                 start=True, stop=False)
                first = False
            nc.tensor.matmul(ph[:], E_bf[:], u_of(2 * k - 2),
                             start=first, stop=False)
            nc.tensor.matmul(ph[:], Min_bf[:], u_of(2 * k - 1),
                             start=False, stop=True)
            h_cur = hp.tile([128, BC], BF16, tag="h", name="h")
            ev_copy(v if k % 2 else s, h_cur[:], ph[:])
            h_tiles[k] = h_cur

        def consumer(j):
            # both chunks of pair j in one 2-bank PSUM tile; T0 terms
            # first so the near field runs before h_j lands (in-bank
            # accumulation executes in program order)
            h_j = h_tiles[j]
            py = py_p.tile([128, 2 * BC], F32, tag="py", name="py")
            if h_j is not None:
                nc.tensor.matmul(py[:, 0:BC], T0_bf[:], u_of(2 * j),
                                 start=True, stop=False)
                nc.tensor.matmul(py[:, BC:2 * BC], T0_bf[:], u_of(2 * j + 1),
                                 start=True, stop=False)
                nc.tensor.matmul(py[:, BC:2 * BC], G0_bf[:], u_of(2 * j),
                                 start=False, stop=False)
                nc.tensor.matmul(py[:, 0:BC], Wout_bf[:], h_j[:],
                                 start=False, stop=True)
                nc.tensor.matmul(py[:, BC:2 * BC], W1_bf[:], h_j[:],
                                 start=False, stop=True)
            else:
                nc.tensor.matmul(py[:, 0:BC], T0_bf[:], u_of(2 * j),
                                 start=True, stop=True)
                nc.tensor.matmul(py[:, BC:2 * BC], T0_bf[:], u_of(2 * j + 1),
                                 start=True, stop=False)
                nc.tensor.matmul(py[:, BC:2 * BC], G0_bf[:], u_of(2 * j),
                                 start=False, stop=True)
            yt_t = yp.tile([128, 2, BC], BF16, tag="ytt", name="ytt")
            ev_copy(s if j % 2 else v, yt_t[:], py[:])
            nc.sync.dma_start(out=yt_r[:, 2 * j:2 * j + 2, :], in_=yt_t[:])

        # trios can pre-run as soon as Min/E/Dq2 are ready
        trio(1); trio(2)

        # ---- alias correction on the C side: Wout'' = (I+X^T+X^2T)Wout
        if BF16_CHAIN:
            Wout_pre_bf = bf("Wout_pre_bf", Wout_f)
            wc1 = mm_ev(A2[NSQ], Wout_pre_bf, 128, 128, "wc1")
            trio(3)
            wc1b = bf("wc1b", wc1)
            wc2 = mm_ev(A2[NSQ], wc1b, 128, 128, "wc2", eng=s)
        else:
            wc1 = mm_ev(A2[NSQ], Wout_f, 128, 128, "wc1")
            trio(3)
            wc2 = mm_ev(A2[NSQ], wc1, 128, 128, "wc2", eng=s)
        v.tensor_add(Wout_f[:], Wout_f[:], wc1[:])
        v.tensor_add(Wout_f[:], Wout_f[:], wc2[:])
        Wout_bf = bf("Wout_bf", Wout_f)
        trio(4)

        # ---- K taps row -> Toeplitz T0 via DRAM shift trick ----------
        psk = pps.tile([128, 128], F32, tag="pp_ps", name="psk")
        nc.tensor.matmul(psk[0:1, 0:128], b2[:], Wout_f[:], start=True, stop=True)
        zK = T([1, 128], tag="zK")
        v.tensor_copy(zK[:], psk[0:1, 0:128])
        v.tensor_add(zK[0:1, 0:1], zK[0:1, 0:1], dval)   # += D at lag 0
        nc.sync.dma_start(out=zs[128:256], in_=zK[:])
        trio(5)
        # T0R[p, t] = zs[1 + p + t] = T0[127-p, t]; un-reverse via rev@T0R
        T0R = T([128, 128], tag="T0R")
        zsap = zs[:]
        src = bass.AP(zsap.tensor, zsap.offset + 1, [[1, 128], [1, 128]])
        nc.sync.dma_start(out=T0R[:], in_=src)
        # remaining far maps while the DMAs fly:
        # W1 = block(Dq)^T @ Wout'' ; G0 = Min @ Wout''
        W1_f = mm_ev(A2_7_f32, Wout_f, 128, 128, "W1_f")
        G0_f = mm_ev(MinT_f, Wout_f, 128, 128, "G0_f", eng=s)
        W1_bf = bf("W1_bf", W1_f)
        G0_bf = bf("G0_bf", G0_f, eng=s)
        trio(6)
        T0f = mm_ev(revm, T0R, 128, 128, "T0f")
        T0_bf = bf("T0_bf", T0f)
        trio(7)

        # ================= main loop ==================================
        nxt = 0
        for it in range(8, NPAIR):
            trio(it)
            consumer(nxt); nxt += 1
        while nxt < NPAIR:
            consumer(nxt); nxt += 1


def kernel(**inputs):
    global LAST_EXEC_NS, LAST_RESULTS
    nc = build_program()
    cmat = _consts()

    u = np.asarray(inputs["u"], dtype=np.float32)
    # per-core pre-transpose to [q, i, (j, b')] and cast to bf16
    # row = c*512 + j*128 + b', col = i*128 + q
    ut = u.reshape(NCORES, 4, 128, NCH, 128).transpose(0, 4, 3, 1, 2)
    ut = np.ascontiguousarray(ut).reshape(NCORES, 128, NCH * BC)
    ut = ut.astype(ml_dtypes.bfloat16)

    par = np.zeros((1, PARW), dtype=np.float32)
    par[0, O_LRE:O_LRE + N] = np.asarray(inputs["Lambda_re"], np.float32)
    par[0, O_LIM:O_LIM + N] = np.asarray(inputs["Lambda_im"], np.float32)
    par[0, O_PRE:O_PRE + N] = np.asarray(inputs["P_re"], np.float32)
    par[0, O_PIM:O_PIM + N] = np.asarray(inputs["P_im"], np.float32)
    par[0, O_BRE:O_BRE + N] = np.asarray(inputs["B_re"], np.float32)
    par[0, O_BIM:O_BIM + N] = np.asarray(inputs["B_im"], np.float32)
    par[0, O_CRE:O_CRE + N] = np.asarray(inputs["C_ri"], np.float32)[:, 0]
    par[0, O_CIM:O_CIM + N] = np.asarray(inputs["C_ri"], np.float32)[:, 1]
    par[0, O_D] = np.asarray(inputs["D"], np.float32).reshape(-1)[0]
    par[0, O_LSTEP] = np.asarray(inputs["log_step"], np.float32).reshape(-1)[0]
    par[0, O_ONE] = 1.0

    in_maps = []
    for c in range(NCORES):
        in_maps.append({"ut": ut[c], "par": par, "cmat": cmat})

    trace = bool(int(os.environ.get("KERNEL_TRACE", "0")))
    kw = {}
    if trace:
        kw["trace"] = True
        kw["trace_cores"] = list(range(NCORES))
    res = run_bass_kernel_spmd(nc, in_maps, list(range(NCORES)), **kw)
    LAST_EXEC_NS = res.exec_time_ns
    LAST_RESULTS = res

    y = np.empty((BH, L), dtype=np.float32)
    for c in range(NCORES):
        ytc = np.asarray(res.results[c]["yt"]).astype(np.float32)  # [i, t, jb]
        y[c * BC:(c + 1) * BC] = (
            ytc.reshape(NCH, 128, 4, 128).transpose(2, 3, 0, 1).reshape(BC, L))
    return y
